# revision 42
# baseline (speedup 1.0000x reference)
"""Trainium2 Bass kernel for nn_Encoder segment-reduce.

Reference computation (per sample b):
    cls = onehot(argmax_k outputs[b])            # [K, HW]
    sizes = cls.sum(HW) + 0.01                   # [K]
    feat_set = feats[b] @ cls.T / sizes          # [F, K]
    out[b] = w_proj @ feat_set + bias            # [E, K]

Kernel strategy (pure data parallel: 1 sample per NeuronCore, 8 cores).

Segment-reduce FIRST (the cheap contraction), projection second:
    feat_setT[k, f] = sum_hw onehot[hw, k] * featsT[hw, f]
computed with the onehot chunk [128hw, 21] as the PE's stationary operand and
featsT chunks [128hw, 512f] as the moving operand, accumulating four [21, 512]
PSUM tiles across all 32 hw chunks.  This streams feats through the PE exactly
once (65K cycles) — the minimum possible — so the kernel is DMA-bound.
A parallel [21, 2] PSUM tile accumulates onehot.T @ ones = the class sizes.

The host supplies:
  - outputs pixel-major [p, t, k] so the argmax is a free-dim reduce (DVE)
    with no PE transposes;
  - featsT block-major [p, t4, fgrp, 512] (a pure layout permutation of the
    bf16-cast feats) so each partition's per-block DMA run is 8KB contiguous.

After the stream: scale rows by 1/sizes, PE-transpose the [21, 2048] result
back to f-major in 128-col chunks, and apply the (tiny) w_proj projection +
bias, writing [E, K] directly.

A burst of dummy matmuls at kernel start keeps the PE's HAM clock gate warm
through the initial DMA window (cold PE runs at 1.2 GHz vs 2.4 GHz warm).

dtype: "bf16" (rel err ~3e-3, half HBM traffic) or "f32r" (float32r full-rate
fp32 matmuls, rel err ~2e-4, double the traffic).
"""

import numpy as np

import concourse.bacc as bacc
import concourse.bass as bass
import concourse.mybir as mybir
import concourse.tile as tile
from concourse.bass import ds, ts
from concourse.bass_utils import run_bass_kernel_spmd
from concourse.masks import make_identity

# Problem shapes (hardcoded per contract)
B = 8
K = 21
H = 64
W = 64
HW = H * W            # 4096
F = 2048
E = 256
P = 128
FC = F // P           # 16 f-chunks of 128
FG = 4                # f-groups of 512 (psum accumulate tiles)
FGW = F // FG         # 512
N_T = HW // P         # 32 hw chunks
TB = 2                # hw chunks per DMA block
N_BLK = N_T // TB     # 8 blocks (2MB bf16 each)
N_CORES = 8

F32 = mybir.dt.float32
F32R = mybir.dt.float32r
BF16 = mybir.dt.bfloat16

DTYPE = "bf16"        # "bf16" or "f32r"


def build_module(dtype=DTYPE, feats_bufs=8, warmup=100):
    mm_dt = BF16 if dtype == "bf16" else F32R
    nc = bacc.Bacc("TRN2", target_bir_lowering=False, debug=False)

    # outputs host-transposed to [p, t, k] (pixel-major).
    outputs_d = nc.dram_tensor("outputs_in", [P, N_T, K], F32, kind="ExternalInput")
    # featsT host-permuted to [p, t, fgrp, fj]: featsT[t*128+p, fgrp*512+fj].
    feats_d = nc.dram_tensor(
        "feats_in", [P, N_T, FG, FGW], mm_dt, kind="ExternalInput"
    )
    wT_d = nc.dram_tensor("wT_in", [F, E], mm_dt, kind="ExternalInput")
    bias_d = nc.dram_tensor("bias_in", [E], F32, kind="ExternalInput")
    out_d = nc.dram_tensor("out", [E, K], F32, kind="ExternalOutput")

    with tile.TileContext(nc) as tc:
        with (
            tc.tile_pool(name="consts", bufs=1) as consts,
            tc.tile_pool(name="feats", bufs=feats_bufs) as feats_pool,
            tc.tile_pool(name="small", bufs=4) as small,
            tc.tile_pool(name="outp", bufs=1) as outp,
            tc.tile_pool(name="ps_fs", bufs=1, space="PSUM") as ps_fs,
            tc.tile_pool(name="ps_sz", bufs=1, space="PSUM") as ps_sz,
            tc.tile_pool(name="ps_misc", bufs=3, space="PSUM") as ps_misc,
        ):
            # Bulk DMAs in FIFO order on the sync HWDGE queue: outputs first
            # (phase 1), then the featsT block stream.  wT/bias ride the
            # gpsimd SWDGE queue in parallel (needed only at the tail).
            # feats blocks alternate between the sync HWDGE queue and the
            # gpsimd SWDGE queue so per-transfer dead time on one queue hides
            # behind the other; outputs/wT/bias slot in early on each queue.
            feats_r = feats_d.ap()
            fgs = []

            def load_block(g):
                fg = feats_pool.tile([P, TB, FG, FGW], mm_dt, name=f"fg{g}",
                                     tag="fg")
                eng = nc.sync if g % 2 == 0 else nc.gpsimd
                eng.dma_start(out=fg, in_=feats_r[:, ds(g * TB, TB)])
                fgs.append(fg)

            wT_sb = consts.tile([P, FC, E], mm_dt)
            nc.gpsimd.dma_start(
                out=wT_sb, in_=wT_d.ap().rearrange("(fc p) e -> p fc e", p=P)
            )
            bias_sb = consts.tile([P, 2], F32)
            nc.gpsimd.dma_start(
                out=bias_sb, in_=bias_d.ap().rearrange("(ec p) -> p ec", p=P)
            )
            load_block(0)
            outputs_sb = consts.tile([P, N_T, K], F32)
            nc.sync.dma_start(out=outputs_sb, in_=outputs_d.ap())
            for g in range(1, N_BLK):
                load_block(g)

            # PE warm-up: HAM holds the PE at 1.2 GHz until ~3.4us of
            # sustained activity; dummy matmuls bridge the initial DMA wait.
            warm_w = consts.tile([P, 64], BF16)
            nc.vector.memset(warm_w, 0.0)
            warm_ps = ps_misc.tile([P, 64], F32, tag="m")
            for _ in range(warmup):
                nc.tensor.matmul(warm_ps[0:64, :], lhsT=warm_w, rhs=warm_w)

            ident = consts.tile([P, P], F32)
            make_identity(nc, ident)
            ones_b = consts.tile([P, 2], mm_dt)
            nc.vector.memset(ones_b, 1.0)

            # Phase 1 (DVE only): onehot = (outT == rowmax) per hw chunk.
            oh_all = consts.tile([P, N_T, K], mm_dt)
            for t in range(N_T):
                rowmax = small.tile([P, 1], F32)
                nc.vector.tensor_reduce(
                    rowmax, outputs_sb[:, t, :], mybir.AxisListType.X,
                    mybir.AluOpType.max,
                )
                nc.vector.tensor_scalar(
                    out=oh_all[:, t, :],
                    in0=outputs_sb[:, t, :],
                    scalar1=rowmax,
                    scalar2=None,
                    op0=mybir.AluOpType.is_equal,
                )

            # Segment-reduce stream: feat_setT[k, f] and the class sizes
            # accumulate in PSUM across all 32 hw chunks; feats passes the
            # PE exactly once.
            fs_ps = [
                ps_fs.tile([K, FGW], F32, name=f"fs{i}", tag=f"fs{i}")
                for i in range(FG)
            ]
            sz_ps = ps_sz.tile([K, 2], F32)
            for g in range(N_BLK):
                fg = fgs[g]
                for ti in range(TB):
                    t = g * TB + ti
                    oh_t = oh_all[:, t, :]
                    for fgrp in range(FG):
                        nc.tensor.matmul(
                            fs_ps[fgrp],
                            lhsT=oh_t,
                            rhs=fg[:, ti, fgrp, :],
                            start=(t == 0),
                            stop=(t == N_T - 1),
                        )
                    nc.tensor.matmul(
                        sz_ps,
                        lhsT=oh_t,
                        rhs=ones_b,
                        start=(t == 0),
                        stop=(t == N_T - 1),
                    )
            sizes_sb = small.tile([K, 1], F32, tag="sizes")
            nc.vector.tensor_scalar_add(sizes_sb, sz_ps[:, 0:1], 0.01)
            recip = small.tile([K, 1], F32, tag="recip")
            nc.vector.reciprocal(recip, sizes_sb)

            # Tail: divide by sizes (fused into the PSUM->SBUF copies, split
            # across DVE and ACT), transpose feat_set back to f-major,
            # project with w_proj, add bias, store [E, K].
            fs_sc = consts.tile([K, F], mm_dt)
            for fgrp in range(FG):
                if fgrp % 2 == 0:
                    nc.vector.tensor_scalar_mul(
                        fs_sc[:, ds(fgrp * FGW, FGW)], fs_ps[fgrp], recip
                    )
                else:
                    nc.scalar.activation(
                        out=fs_sc[:, ds(fgrp * FGW, FGW)],
                        in_=fs_ps[fgrp],
                        func=mybir.ActivationFunctionType.Copy,
                        scale=recip,
                    )

            ident_b = consts.tile([K, K], mm_dt)
            nc.vector.tensor_copy(ident_b, ident[:K, :K])
            fsT_sb = consts.tile([P, FC, K], mm_dt)
            ps_o = [None, None]
            out_sb = outp.tile([P, 2, K], F32)
            for ec in range(2):
                ps_o_ec = ps_misc.tile([P, K], F32, tag="m", name=f"ps_o{ec}")
                ps_o[ec] = ps_o_ec
            for fc in range(FC):
                # trp reuses the ps_fs slots (free once the scales are done),
                # giving the transpose->copy chain a 4-deep pipeline.
                trp = ps_fs.tile(
                    [P, K], mm_dt, name=f"trp{fc}", tag=f"fs{fc % FG}"
                )
                nc.tensor.transpose(trp, fs_sc[:, ts(fc, P)], ident_b)
                nc.vector.tensor_copy(fsT_sb[:, fc, :], trp)
                for ec in range(2):
                    nc.tensor.matmul(
                        ps_o[ec],
                        lhsT=wT_sb[:, fc, ds(ec * P, P)],
                        rhs=fsT_sb[:, fc, :],
                        start=(fc == 0),
                        stop=(fc == FC - 1),
                    )
            for ec in range(2):
                nc.vector.tensor_scalar_add(
                    out_sb[:, ec, :], ps_o[ec], bias_sb[:, ec : ec + 1]
                )
            nc.sync.dma_start(
                out=out_d.ap().rearrange("(ec p) k -> p ec k", p=P), in_=out_sb
            )

    nc.compile()
    return nc


_CACHE = {}


def make_in_maps(outputs, feats, w_proj, b_proj, dtype=DTYPE):
    import ml_dtypes

    mm_np = ml_dtypes.bfloat16 if dtype == "bf16" else np.float32
    outputs = np.asarray(outputs, dtype=np.float32)
    # [B, K, H, W] -> per sample [p, t, k] (pixel-major: hw = t*128 + p)
    outputs_t = np.ascontiguousarray(
        outputs.reshape(B, K, N_T, P).transpose(0, 3, 2, 1)
    )
    feats = np.asarray(feats, dtype=np.float32).astype(mm_np)
    # [B, F, H, W] -> per sample [p, t, fgrp, fj] = featsT[t*128+p, fgrp*512+fj]
    feats_sh = np.ascontiguousarray(
        feats.reshape(B, FG, FGW, N_T, P).transpose(0, 4, 3, 1, 2)
    )
    wT = np.ascontiguousarray(np.asarray(w_proj, dtype=np.float32).T.astype(mm_np))
    bias = np.ascontiguousarray(np.asarray(b_proj, dtype=np.float32))
    return [
        {
            "outputs_in": outputs_t[b],
            "feats_in": feats_sh[b],
            "wT_in": wT,
            "bias_in": bias,
        }
        for b in range(B)
    ]


def kernel(outputs, feats, w_proj, b_proj, _trace=False, _trace_kwargs=None,
           _dtype=DTYPE, _build_kwargs=None):
    key = (_dtype, tuple(sorted((_build_kwargs or {}).items())))
    if key not in _CACHE:
        _CACHE[key] = build_module(dtype=_dtype, **(_build_kwargs or {}))
    nc = _CACHE[key]
    in_maps = make_in_maps(outputs, feats, w_proj, b_proj, dtype=_dtype)
    res = run_bass_kernel_spmd(
        nc,
        in_maps,
        core_ids=list(range(N_CORES)),
        trace=_trace,
        **(_trace_kwargs or {}),
    )
    out = np.stack([np.asarray(r["out"]) for r in res.results])
    if _trace:
        _CACHE["last_results"] = res
    return out


# revision 43
# speedup vs baseline: 1.0792x; 1.0792x over previous
"""Trainium2 Bass kernel for nn_Encoder segment-reduce.

Reference computation (per sample b):
    cls = onehot(argmax_k outputs[b])            # [K, HW]
    sizes = cls.sum(HW) + 0.01                   # [K]
    feat_set = feats[b] @ cls.T / sizes          # [F, K]
    out[b] = w_proj @ feat_set + bias            # [E, K]

Kernel strategy (pure data parallel: 1 sample per NeuronCore, 8 cores).

Segment-reduce FIRST (the cheap contraction), projection second:
    feat_setT[k, f] = sum_hw onehot[hw, k] * featsT[hw, f]
computed with the onehot chunk [128hw, 21] as the PE's stationary operand and
featsT chunks [128hw, 512f] as the moving operand, accumulating four [21, 512]
PSUM tiles across all 32 hw chunks.  This streams feats through the PE exactly
once (65K cycles) — the minimum possible — so the kernel is DMA-bound.
A parallel [21, 2] PSUM tile accumulates onehot.T @ ones = the class sizes.

The host supplies:
  - outputs pixel-major [p, t, k] so the argmax is a free-dim reduce (DVE)
    with no PE transposes;
  - featsT block-major [p, t4, fgrp, 512] (a pure layout permutation of the
    bf16-cast feats) so each partition's per-block DMA run is 8KB contiguous.

After the stream: scale rows by 1/sizes, PE-transpose the [21, 2048] result
back to f-major in 128-col chunks, and apply the (tiny) w_proj projection +
bias, writing [E, K] directly.

A burst of dummy matmuls at kernel start keeps the PE's HAM clock gate warm
through the initial DMA window (cold PE runs at 1.2 GHz vs 2.4 GHz warm).

dtype: "bf16" (rel err ~3e-3, half HBM traffic) or "f32r" (float32r full-rate
fp32 matmuls, rel err ~2e-4, double the traffic).
"""

import numpy as np

import concourse.bacc as bacc
import concourse.bass as bass
import concourse.mybir as mybir
import concourse.tile as tile
from concourse.bass import ds, ts
from concourse.bass_utils import run_bass_kernel_spmd
from concourse.masks import make_identity

# Problem shapes (hardcoded per contract)
B = 8
K = 21
H = 64
W = 64
HW = H * W            # 4096
F = 2048
E = 256
P = 128
FC = F // P           # 16 f-chunks of 128
FG = 4                # f-groups of 512 (psum accumulate tiles)
FGW = F // FG         # 512
N_T = HW // P         # 32 hw chunks
TB = 2                # hw chunks per DMA block
N_BLK = N_T // TB     # 8 blocks (2MB bf16 each)
N_CORES = 8

F32 = mybir.dt.float32
F32R = mybir.dt.float32r
BF16 = mybir.dt.bfloat16

DTYPE = "bf16"        # "bf16" or "f32r"


def build_module(dtype=DTYPE, feats_bufs=8, warmup=100):
    mm_dt = BF16 if dtype == "bf16" else F32R
    nc = bacc.Bacc("TRN2", target_bir_lowering=False, debug=False)

    # outputs host-transposed to [p, t, k] (pixel-major).
    outputs_d = nc.dram_tensor("outputs_in", [P, N_T, K], F32, kind="ExternalInput")
    # featsT host-permuted to [p, t, fgrp, fj]: featsT[t*128+p, fgrp*512+fj].
    feats_d = nc.dram_tensor(
        "feats_in", [P, N_T, FG, FGW], mm_dt, kind="ExternalInput"
    )
    wT_d = nc.dram_tensor("wT_in", [F, E], mm_dt, kind="ExternalInput")
    bias_d = nc.dram_tensor("bias_in", [E], F32, kind="ExternalInput")
    out_d = nc.dram_tensor("out", [E, K], F32, kind="ExternalOutput")

    with tile.TileContext(nc) as tc:
        with (
            tc.tile_pool(name="consts", bufs=1) as consts,
            tc.tile_pool(name="feats", bufs=feats_bufs) as feats_pool,
            tc.tile_pool(name="small", bufs=4) as small,
            tc.tile_pool(name="outp", bufs=1) as outp,
            tc.tile_pool(name="ps_fs", bufs=1, space="PSUM") as ps_fs,
            tc.tile_pool(name="ps_sz", bufs=1, space="PSUM") as ps_sz,
            tc.tile_pool(name="ps_misc", bufs=3, space="PSUM") as ps_misc,
        ):
            # Bulk DMAs in FIFO order on the sync HWDGE queue: outputs first
            # (phase 1), then the featsT block stream.  wT/bias ride the
            # gpsimd SWDGE queue in parallel (needed only at the tail).
            # feats blocks alternate between the sync HWDGE queue and the
            # gpsimd SWDGE queue so per-transfer dead time on one queue hides
            # behind the other; outputs/wT/bias slot in early on each queue.
            feats_r = feats_d.ap()
            fgs = []

            def load_block(g):
                fg = feats_pool.tile([P, TB, FG, FGW], mm_dt, name=f"fg{g}",
                                     tag="fg")
                nc.sync.dma_start(out=fg, in_=feats_r[:, ds(g * TB, TB)])
                fgs.append(fg)

            wT_sb = consts.tile([P, FC, E], mm_dt)
            nc.gpsimd.dma_start(
                out=wT_sb, in_=wT_d.ap().rearrange("(fc p) e -> p fc e", p=P)
            )
            bias_sb = consts.tile([P, 2], F32)
            nc.gpsimd.dma_start(
                out=bias_sb, in_=bias_d.ap().rearrange("(ec p) -> p ec", p=P)
            )
            load_block(0)
            outputs_sb = consts.tile([P, N_T, K], F32)
            nc.sync.dma_start(out=outputs_sb, in_=outputs_d.ap())
            for g in range(1, N_BLK):
                load_block(g)

            # PE warm-up: HAM holds the PE at 1.2 GHz until ~3.4us of
            # sustained activity; dummy matmuls bridge the initial DMA wait.
            warm_w = consts.tile([P, 64], BF16)
            nc.vector.memset(warm_w, 0.0)
            warm_ps = ps_misc.tile([P, 64], F32, tag="m")
            for _ in range(warmup):
                nc.tensor.matmul(warm_ps[0:64, :], lhsT=warm_w, rhs=warm_w)

            ident = consts.tile([P, P], F32)
            make_identity(nc, ident)
            ones_b = consts.tile([P, 2], mm_dt)
            nc.vector.memset(ones_b, 1.0)

            # Phase 1 (DVE only): onehot = (outT == rowmax) per hw chunk.
            oh_all = consts.tile([P, N_T, K], mm_dt)
            for t in range(N_T):
                rowmax = small.tile([P, 1], F32)
                nc.vector.tensor_reduce(
                    rowmax, outputs_sb[:, t, :], mybir.AxisListType.X,
                    mybir.AluOpType.max,
                )
                nc.vector.tensor_scalar(
                    out=oh_all[:, t, :],
                    in0=outputs_sb[:, t, :],
                    scalar1=rowmax,
                    scalar2=None,
                    op0=mybir.AluOpType.is_equal,
                )

            # Segment-reduce stream: feat_setT[k, f] and the class sizes
            # accumulate in PSUM across all 32 hw chunks; feats passes the
            # PE exactly once.
            fs_ps = [
                ps_fs.tile([K, FGW], F32, name=f"fs{i}", tag=f"fs{i}")
                for i in range(FG)
            ]
            sz_ps = ps_sz.tile([K, 2], F32)
            for g in range(N_BLK):
                fg = fgs[g]
                for ti in range(TB):
                    t = g * TB + ti
                    oh_t = oh_all[:, t, :]
                    for fgrp in range(FG):
                        nc.tensor.matmul(
                            fs_ps[fgrp],
                            lhsT=oh_t,
                            rhs=fg[:, ti, fgrp, :],
                            start=(t == 0),
                            stop=(t == N_T - 1),
                        )
                    nc.tensor.matmul(
                        sz_ps,
                        lhsT=oh_t,
                        rhs=ones_b,
                        start=(t == 0),
                        stop=(t == N_T - 1),
                    )
            sizes_sb = small.tile([K, 1], F32, tag="sizes")
            nc.vector.tensor_scalar_add(sizes_sb, sz_ps[:, 0:1], 0.01)
            recip = small.tile([K, 1], F32, tag="recip")
            nc.vector.reciprocal(recip, sizes_sb)

            # Tail: divide by sizes (fused into the PSUM->SBUF copies, split
            # across DVE and ACT), transpose feat_set back to f-major,
            # project with w_proj, add bias, store [E, K].
            fs_sc = consts.tile([K, F], mm_dt)
            for fgrp in range(FG):
                if fgrp % 2 == 0:
                    nc.vector.tensor_scalar_mul(
                        fs_sc[:, ds(fgrp * FGW, FGW)], fs_ps[fgrp], recip
                    )
                else:
                    nc.scalar.activation(
                        out=fs_sc[:, ds(fgrp * FGW, FGW)],
                        in_=fs_ps[fgrp],
                        func=mybir.ActivationFunctionType.Copy,
                        scale=recip,
                    )

            ident_b = consts.tile([K, K], mm_dt)
            nc.vector.tensor_copy(ident_b, ident[:K, :K])
            fsT_sb = consts.tile([P, FC, K], mm_dt)
            ps_o = [None, None]
            out_sb = outp.tile([P, 2, K], F32)
            for ec in range(2):
                ps_o_ec = ps_misc.tile([P, K], F32, tag="m", name=f"ps_o{ec}")
                ps_o[ec] = ps_o_ec
            for fc in range(FC):
                # trp reuses the ps_fs slots (free once the scales are done),
                # giving the transpose->copy chain a 4-deep pipeline.
                trp = ps_fs.tile(
                    [P, K], mm_dt, name=f"trp{fc}", tag=f"fs{fc % FG}"
                )
                nc.tensor.transpose(trp, fs_sc[:, ts(fc, P)], ident_b)
                nc.vector.tensor_copy(fsT_sb[:, fc, :], trp)
                for ec in range(2):
                    nc.tensor.matmul(
                        ps_o[ec],
                        lhsT=wT_sb[:, fc, ds(ec * P, P)],
                        rhs=fsT_sb[:, fc, :],
                        start=(fc == 0),
                        stop=(fc == FC - 1),
                    )
            for ec in range(2):
                nc.vector.tensor_scalar_add(
                    out_sb[:, ec, :], ps_o[ec], bias_sb[:, ec : ec + 1]
                )
            nc.sync.dma_start(
                out=out_d.ap().rearrange("(ec p) k -> p ec k", p=P), in_=out_sb
            )

    nc.compile()
    return nc


_CACHE = {}


def make_in_maps(outputs, feats, w_proj, b_proj, dtype=DTYPE):
    import ml_dtypes

    mm_np = ml_dtypes.bfloat16 if dtype == "bf16" else np.float32
    outputs = np.asarray(outputs, dtype=np.float32)
    # [B, K, H, W] -> per sample [p, t, k] (pixel-major: hw = t*128 + p)
    outputs_t = np.ascontiguousarray(
        outputs.reshape(B, K, N_T, P).transpose(0, 3, 2, 1)
    )
    feats = np.asarray(feats, dtype=np.float32).astype(mm_np)
    # [B, F, H, W] -> per sample [p, t, fgrp, fj] = featsT[t*128+p, fgrp*512+fj]
    feats_sh = np.ascontiguousarray(
        feats.reshape(B, FG, FGW, N_T, P).transpose(0, 4, 3, 1, 2)
    )
    wT = np.ascontiguousarray(np.asarray(w_proj, dtype=np.float32).T.astype(mm_np))
    bias = np.ascontiguousarray(np.asarray(b_proj, dtype=np.float32))
    return [
        {
            "outputs_in": outputs_t[b],
            "feats_in": feats_sh[b],
            "wT_in": wT,
            "bias_in": bias,
        }
        for b in range(B)
    ]


def kernel(outputs, feats, w_proj, b_proj, _trace=False, _trace_kwargs=None,
           _dtype=DTYPE, _build_kwargs=None):
    key = (_dtype, tuple(sorted((_build_kwargs or {}).items())))
    if key not in _CACHE:
        _CACHE[key] = build_module(dtype=_dtype, **(_build_kwargs or {}))
    nc = _CACHE[key]
    in_maps = make_in_maps(outputs, feats, w_proj, b_proj, dtype=_dtype)
    res = run_bass_kernel_spmd(
        nc,
        in_maps,
        core_ids=list(range(N_CORES)),
        trace=_trace,
        **(_trace_kwargs or {}),
    )
    out = np.stack([np.asarray(r["out"]) for r in res.results])
    if _trace:
        _CACHE["last_results"] = res
    return out


# revision 44
# speedup vs baseline: 1.1585x; 1.0735x over previous
"""Trainium2 Bass kernel for nn_Encoder segment-reduce.

Reference computation (per sample b):
    cls = onehot(argmax_k outputs[b])            # [K, HW]
    sizes = cls.sum(HW) + 0.01                   # [K]
    feat_set = feats[b] @ cls.T / sizes          # [F, K]
    out[b] = w_proj @ feat_set + bias            # [E, K]

Kernel strategy (pure data parallel: 1 sample per NeuronCore, 8 cores).

Segment-reduce FIRST (the cheap contraction), projection second:
    feat_setT[k, f] = sum_hw onehot[hw, k] * featsT[hw, f]
computed with the onehot chunk [128hw, 21] as the PE's stationary operand and
featsT chunks [128hw, 512f] as the moving operand, accumulating four [21, 512]
PSUM tiles across all 32 hw chunks.  This streams feats through the PE exactly
once (65K cycles) — the minimum possible — so the kernel is DMA-bound.
A parallel [21, 2] PSUM tile accumulates onehot.T @ ones = the class sizes.

The host supplies:
  - outputs pixel-major [p, t, k] so the argmax is a free-dim reduce (DVE)
    with no PE transposes;
  - featsT block-major [p, t4, fgrp, 512] (a pure layout permutation of the
    bf16-cast feats) so each partition's per-block DMA run is 8KB contiguous.

After the stream: scale rows by 1/sizes, PE-transpose the [21, 2048] result
back to f-major in 128-col chunks, and apply the (tiny) w_proj projection +
bias, writing [E, K] directly.

A burst of dummy matmuls at kernel start keeps the PE's HAM clock gate warm
through the initial DMA window (cold PE runs at 1.2 GHz vs 2.4 GHz warm).

dtype: "bf16" (rel err ~3e-3, half HBM traffic) or "f32r" (float32r full-rate
fp32 matmuls, rel err ~2e-4, double the traffic).
"""

import numpy as np

import concourse.bacc as bacc
import concourse.bass as bass
import concourse.mybir as mybir
import concourse.tile as tile
from concourse.bass import ds, ts
from concourse.bass_utils import run_bass_kernel_spmd
from concourse.masks import make_identity

# Problem shapes (hardcoded per contract)
B = 8
K = 21
H = 64
W = 64
HW = H * W            # 4096
F = 2048
E = 256
P = 128
FC = F // P           # 16 f-chunks of 128
FG = 4                # f-groups of 512 (psum accumulate tiles)
FGW = F // FG         # 512
N_T = HW // P         # 32 hw chunks
TB = 2                # hw chunks per DMA block
N_BLK = N_T // TB     # 8 blocks (2MB bf16 each)
N_CORES = 8

F32 = mybir.dt.float32
F32R = mybir.dt.float32r
BF16 = mybir.dt.bfloat16

DTYPE = "bf16"        # "bf16" or "f32r"


def build_module(dtype=DTYPE, feats_bufs=8, warmup=100):
    mm_dt = BF16 if dtype == "bf16" else F32R
    nc = bacc.Bacc("TRN2", target_bir_lowering=False, debug=False)

    # outputs host-transposed to [p, t, k] (pixel-major).
    outputs_d = nc.dram_tensor("outputs_in", [P, N_T, K], F32, kind="ExternalInput")
    # featsT host-permuted to [p, t, fgrp, fj]: featsT[t*128+p, fgrp*512+fj].
    feats_d = nc.dram_tensor(
        "feats_in", [P, N_T, FG, FGW], mm_dt, kind="ExternalInput"
    )
    wT_d = nc.dram_tensor("wT_in", [F, E], mm_dt, kind="ExternalInput")
    bias_d = nc.dram_tensor("bias_in", [E], F32, kind="ExternalInput")
    out_d = nc.dram_tensor("out", [E, K], F32, kind="ExternalOutput")

    with tile.TileContext(nc) as tc:
        with (
            tc.tile_pool(name="consts", bufs=1) as consts,
            tc.tile_pool(name="feats", bufs=feats_bufs) as feats_pool,
            tc.tile_pool(name="small", bufs=4) as small,
            tc.tile_pool(name="outp", bufs=1) as outp,
            tc.tile_pool(name="ps_fs", bufs=1, space="PSUM") as ps_fs,
            tc.tile_pool(name="ps_sz", bufs=1, space="PSUM") as ps_sz,
            tc.tile_pool(name="ps_misc", bufs=3, space="PSUM") as ps_misc,
        ):
            # Bulk DMAs in FIFO order on the sync HWDGE queue: outputs first
            # (phase 1), then the featsT block stream.  wT/bias ride the
            # gpsimd SWDGE queue in parallel (needed only at the tail).
            # feats blocks alternate between the sync HWDGE queue and the
            # gpsimd SWDGE queue so per-transfer dead time on one queue hides
            # behind the other; outputs/wT/bias slot in early on each queue.
            feats_r = feats_d.ap()
            fgs = []

            def load_block(g):
                fg = feats_pool.tile([P, TB, FG, FGW], mm_dt, name=f"fg{g}",
                                     tag="fg")
                nc.sync.dma_start(out=fg, in_=feats_r[:, ds(g * TB, TB)])
                fgs.append(fg)

            wT_sb = consts.tile([P, FC, E], mm_dt)
            nc.gpsimd.dma_start(
                out=wT_sb, in_=wT_d.ap().rearrange("(fc p) e -> p fc e", p=P)
            )
            bias_sb = consts.tile([P, 2], F32)
            nc.gpsimd.dma_start(
                out=bias_sb, in_=bias_d.ap().rearrange("(ec p) -> p ec", p=P)
            )
            load_block(0)
            outputs_sb = consts.tile([P, N_T, K], F32)
            nc.sync.dma_start(out=outputs_sb, in_=outputs_d.ap())
            for g in range(1, N_BLK):
                load_block(g)

            # PE warm-up: HAM holds the PE at 1.2 GHz until ~3.4us of
            # sustained activity; dummy matmuls bridge the initial DMA wait.
            warm_w = consts.tile([P, 64], BF16)
            nc.vector.memset(warm_w, 0.0)
            warm_ps = ps_misc.tile([P, 64], F32, tag="m")
            for _ in range(warmup):
                nc.tensor.matmul(warm_ps[0:64, :], lhsT=warm_w, rhs=warm_w)

            ident = consts.tile([P, P], F32)
            make_identity(nc, ident)
            ones_b = consts.tile([P, 2], mm_dt)
            nc.vector.memset(ones_b, 1.0)

            # Phase 1 (DVE only): onehot = (outT == rowmax) per hw chunk.
            oh_all = consts.tile([P, N_T, K], mm_dt)
            for t in range(N_T):
                rowmax = small.tile([P, 1], F32)
                nc.vector.tensor_reduce(
                    rowmax, outputs_sb[:, t, :], mybir.AxisListType.X,
                    mybir.AluOpType.max,
                )
                nc.vector.tensor_scalar(
                    out=oh_all[:, t, :],
                    in0=outputs_sb[:, t, :],
                    scalar1=rowmax,
                    scalar2=None,
                    op0=mybir.AluOpType.is_equal,
                )

            # Segment-reduce stream: feat_setT[k, f] and the class sizes
            # accumulate in PSUM across all 32 hw chunks; feats passes the
            # PE exactly once.
            fs_ps = [
                ps_fs.tile([K, FGW], F32, name=f"fs{i}", tag=f"fs{i}")
                for i in range(FG)
            ]
            # The sizes matmuls (only need oh) are packed into the first half
            # of the stream so the reciprocal is ready before the stream ends.
            sz_ps = ps_sz.tile([K, 2], F32)
            recip_emitted = False
            for g in range(N_BLK):
                fg = fgs[g]
                for ti in range(TB):
                    t = g * TB + ti
                    oh_t = oh_all[:, t, :]
                    for fgrp in range(FG):
                        nc.tensor.matmul(
                            fs_ps[fgrp],
                            lhsT=oh_t,
                            rhs=fg[:, ti, fgrp, :],
                            start=(t == 0),
                            stop=(t == N_T - 1),
                        )
                if g < 8:
                    for tz in range(g * 4, g * 4 + 4):
                        nc.tensor.matmul(
                            sz_ps,
                            lhsT=oh_all[:, tz, :],
                            rhs=ones_b,
                            start=(tz == 0),
                            stop=(tz == N_T - 1),
                        )
                elif not recip_emitted:
                    recip_emitted = True
                    sizes_sb = small.tile([K, 1], F32, tag="sizes")
                    nc.vector.tensor_scalar_add(sizes_sb, sz_ps[:, 0:1], 0.01)
                    recip = small.tile([K, 1], F32, tag="recip")
                    nc.vector.reciprocal(recip, sizes_sb)

            # Tail: divide by sizes (fused into the PSUM->SBUF copies, split
            # across DVE and ACT), transpose feat_set back to f-major,
            # project with w_proj, add bias, store [E, K].
            fs_sc = consts.tile([K, F], mm_dt)
            for fgrp in range(FG):
                if fgrp % 2 == 0:
                    nc.vector.tensor_scalar_mul(
                        fs_sc[:, ds(fgrp * FGW, FGW)], fs_ps[fgrp], recip
                    )
                else:
                    nc.scalar.activation(
                        out=fs_sc[:, ds(fgrp * FGW, FGW)],
                        in_=fs_ps[fgrp],
                        func=mybir.ActivationFunctionType.Copy,
                        scale=recip,
                    )

            ident_b = consts.tile([K, K], mm_dt)
            nc.vector.tensor_copy(ident_b, ident[:K, :K])
            fsT_sb = consts.tile([P, FC, K], mm_dt)
            ps_o = [None, None]
            out_sb = outp.tile([P, 2, K], F32)
            for ec in range(2):
                ps_o_ec = ps_misc.tile([P, K], F32, tag="m", name=f"ps_o{ec}")
                ps_o[ec] = ps_o_ec
            for fc in range(FC):
                # trp reuses the ps_fs slots (free once the scales are done),
                # giving the transpose->copy chain a 4-deep pipeline.
                trp = ps_fs.tile(
                    [P, K], mm_dt, name=f"trp{fc}", tag=f"fs{fc % FG}"
                )
                nc.tensor.transpose(trp, fs_sc[:, ts(fc, P)], ident_b)
                nc.vector.tensor_copy(fsT_sb[:, fc, :], trp)
                for ec in range(2):
                    nc.tensor.matmul(
                        ps_o[ec],
                        lhsT=wT_sb[:, fc, ds(ec * P, P)],
                        rhs=fsT_sb[:, fc, :],
                        start=(fc == 0),
                        stop=(fc == FC - 1),
                    )
            for ec in range(2):
                nc.vector.tensor_scalar_add(
                    out_sb[:, ec, :], ps_o[ec], bias_sb[:, ec : ec + 1]
                )
            nc.sync.dma_start(
                out=out_d.ap().rearrange("(ec p) k -> p ec k", p=P), in_=out_sb
            )

    nc.compile()
    return nc


_CACHE = {}


def make_in_maps(outputs, feats, w_proj, b_proj, dtype=DTYPE):
    import ml_dtypes

    mm_np = ml_dtypes.bfloat16 if dtype == "bf16" else np.float32
    outputs = np.asarray(outputs, dtype=np.float32)
    # [B, K, H, W] -> per sample [p, t, k] (pixel-major: hw = t*128 + p)
    outputs_t = np.ascontiguousarray(
        outputs.reshape(B, K, N_T, P).transpose(0, 3, 2, 1)
    )
    feats = np.asarray(feats, dtype=np.float32).astype(mm_np)
    # [B, F, H, W] -> per sample [p, t, fgrp, fj] = featsT[t*128+p, fgrp*512+fj]
    feats_sh = np.ascontiguousarray(
        feats.reshape(B, FG, FGW, N_T, P).transpose(0, 4, 3, 1, 2)
    )
    wT = np.ascontiguousarray(np.asarray(w_proj, dtype=np.float32).T.astype(mm_np))
    bias = np.ascontiguousarray(np.asarray(b_proj, dtype=np.float32))
    return [
        {
            "outputs_in": outputs_t[b],
            "feats_in": feats_sh[b],
            "wT_in": wT,
            "bias_in": bias,
        }
        for b in range(B)
    ]


def kernel(outputs, feats, w_proj, b_proj, _trace=False, _trace_kwargs=None,
           _dtype=DTYPE, _build_kwargs=None):
    key = (_dtype, tuple(sorted((_build_kwargs or {}).items())))
    if key not in _CACHE:
        _CACHE[key] = build_module(dtype=_dtype, **(_build_kwargs or {}))
    nc = _CACHE[key]
    in_maps = make_in_maps(outputs, feats, w_proj, b_proj, dtype=_dtype)
    res = run_bass_kernel_spmd(
        nc,
        in_maps,
        core_ids=list(range(N_CORES)),
        trace=_trace,
        **(_trace_kwargs or {}),
    )
    out = np.stack([np.asarray(r["out"]) for r in res.results])
    if _trace:
        _CACHE["last_results"] = res
    return out
